# revision 24
# baseline (speedup 1.0000x reference)
"""Trainium2 Bass kernel for the sparse-attention (local 3x3 unfold) problem.

Semantics (per channel; the reference's .reshape is a RAW reinterpretation
of [9, L] -> [L, 9], so with RAW = patch-major unfold flattened [9*L]):
  out1_flat[n] = RAW_k[n] * RAW_q[9*(n//9) + 4]     (n in [0, 9L))
  out2_flat[n] = RAW_q[n] * RAW_k[9*(n//9) + 4]

Key identity (v10): RAW[n] == RAW[n + 49024] == RAW[n + 98048] wherever
the patch-row shift stays in range (shifting patch p by +3 keeps the same
column-variant dj while di absorbs the row shift).  So the device loads
only the FIRST THIRD of RAW (~6.3 MB/core instead of 18.9 MB) and each
partition computes all three thirds from one tile via three shifted
access patterns:

  partition s owns spans  n in [start_s + j*49023, + 3078), j = 0,1,2
  with start_s = 3072*s - (3*s mod 9)  (multiple of 9, uniform T=3078,
  tiny overlaps), tile[x] = RAW[start_s - 2 + x] (x in [0,3080)):
    in0 tile-offset = (2-j) + t ;  center tile-offset = (6-j) + 9*(t//9)

The ~0.8% of outputs where the identity breaks (patch rows i=127/126,
center spill, coverage gaps) are computed exactly in f32 on the host
during unscramble (precomputed index masks, channel-independent).

All device traffic is bf16 (gate is 2e-2; bf16 in+product is ~0.6%).
DMA: ~6.3 MB loads + ~19 MB stores per core, all [[W,128],[1,W]] blocks
with 6.2 KB descriptors; vector engine (~81 us, 1x mode) is the wall.

Sharding: pure data-parallel over the 256 (b,c) channels; 32 per core.
"""

import sys

for _p in ("/opt/trn_rl_repo", "/opt/pypackages"):
    if _p not in sys.path:
        sys.path.insert(0, _p)

import numpy as np

import concourse.bass as bass
import concourse.mybir as mybir
import concourse.tile as tile
from concourse.bass import AP
from concourse.bass_utils import run_bass_kernel_spmd
from concourse.vector_clock import ScopedClock

# ---------------------------------------------------------------------------
# Patch: this container's walrus rejects >1 sync-wait on the Tile tail Drain
# ("Too many sync wait commands").  Spill extra waits onto SP NOPs, which
# execute in program order before the all-engine barrier, preserving the
# "all work done before sem clear" semantics.
# ---------------------------------------------------------------------------


def _drain_and_barrier(self, tick_clock, wait_clock):
    nc = self.nc
    drain_inst = nc.sync.drain()
    wait_clock.add_sem_waits(
        drain_inst.ins, ScopedClock({None: tick_clock.global_clock})
    )
    si = drain_inst.ins.sync_info
    if si is not None and len(si.on_wait) > 1:
        waits = list(si.on_wait)
        drain_inst.ins.sync_info = mybir.SyncInfo(
            on_wait=waits[:1], on_update=list(si.on_update)
        )
        for w in waits[1:]:
            nop = nc.sync.nop(nofuse=True)
            nop.ins.sync_info = mybir.SyncInfo(on_wait=[w], on_update=[])

    nc.all_engine_barrier()
    assert self.sems is not None
    popped = nc._tile_sem_poison_stack.pop()
    assert popped is self._sem_poison
    nc.clear_and_free_semaphores(list(self.sems.allocated().values()))
    nc.all_engine_barrier()


tile.TileContext._drain_and_barrier = _drain_and_barrier


def _split_waits(nc, maxw=1):
    """Walrus here allows only `maxw` sync-waits per instruction: move extra
    waits onto same-engine NOPs inserted immediately before the instruction
    (same engine stream => executes before it)."""
    for fn in nc.m.functions:
        for bb in fn.blocks:
            out = []
            for inst in bb.instructions:
                si = getattr(inst, "sync_info", None)
                if si is not None and len(si.on_wait) > maxw:
                    waits = list(si.on_wait)
                    for w in waits[:-maxw]:
                        nop = mybir.InstNoOp(
                            name=nc.get_next_instruction_name(),
                            bass_nofuse=True,
                        )
                        nop.engine = inst.engine
                        nop.sync_info = mybir.SyncInfo(on_wait=[w], on_update=[])
                        nc.register_instruction(nop)
                        out.append(nop)
                    inst.sync_info = mybir.SyncInfo(
                        on_wait=waits[-maxw:], on_update=list(si.on_update)
                    )
                out.append(inst)
            bb.instructions[:] = out

# ---------------------------------------------------------------------------

F32 = mybir.dt.float32
BF16 = mybir.dt.bfloat16
NP_BF16 = mybir.dt.np(mybir.dt.bfloat16)

N_CORES = 8
B, C, H, W = 4, 64, 128, 128
BC = B * C                # 256 channels
CPC = BC // N_CORES       # 32 channels per core
NG = 4                    # channel groups per core
NCH = CPC // NG           # 8 channels per group
SPC = 16                  # partitions per channel
L = H * W
L9 = 9 * L                # 147456 flats per channel
DD = 49024                # RAW periodicity: RAW[n] == RAW[n+DD] (in range)
T3 = 3078                 # span length (multiple of 9)
IT = 3080                 # input tile width (span + 2-elem halo)
OF3 = 3 * T3              # 9234: output tile width (3 spans)
STARTS = [3072 * s - ((3 * s) % 9) for s in range(SPC)]
OFFJ = [0, DD - 1, 2 * DD - 2]   # span offsets (multiples of 9)


def _build_program():
    nc = bass.Bass(trn_type="TRN2")
    kp = nc.dram_tensor("kp", [NG, 128, IT], BF16, kind="ExternalInput")
    qp = nc.dram_tensor("qp", [NG, 128, IT], BF16, kind="ExternalInput")
    o1 = nc.dram_tensor("o1", [NG, 128, OF3], BF16, kind="ExternalOutput")
    o2 = nc.dram_tensor("o2", [NG, 128, OF3], BF16, kind="ExternalOutput")

    # DMA on the two HWDGE queues (16 fat instructions, ~31us DGE each);
    # gpsimd computes one multiply per group (issued first, hidden under
    # the DVE chain), cutting the DVE wall from ~81us to ~67us.
    engines = [nc.sync, nc.scalar]
    eng_i = [0]

    def eng():
        e = engines[eng_i[0] % len(engines)]
        eng_i[0] += 1
        return e

    lblk = [[IT, 128], [1, IT]]
    sblk = [[OF3, 128], [1, OF3]]
    full = [[OF3, 128], [9, T3 // 9], [1, 9]]
    infull = [[IT, 128], [9, T3 // 9], [1, 9]]
    bcast = [[IT, 128], [9, T3 // 9], [0, 9]]
    with tile.TileContext(nc) as tc:
        with (
            tc.tile_pool(name="tin", bufs=3) as tin,
            tc.tile_pool(name="tout", bufs=2) as tout,
        ):
            HSPLIT = 1552             # g0 load split point (free dim)
            FDL = 1539                # left-piece mul FD (= 171*9)
            h2 = [[OF3, 128], [1, 2 * T3]]
            t1 = [[OF3, 128], [1, T3]]
            for g in range(NG):
                tk = tin.tile([128, IT], BF16, tag="tk")
                tq = tin.tile([128, IT], BF16, tag="tq")
                for srcd, t in ((kp, tk), (qp, tq)):
                    if g == 0:
                        # split the very first loads so the first mul can
                        # start on the left half-tile ~3us earlier
                        for lo, hi in ((0, HSPLIT), (HSPLIT, IT)):
                            seg = [[IT, 128], [1, hi - lo]]
                            eng().dma_start(
                                AP(t[:].tensor, lo, seg),
                                AP(srcd, g * 128 * IT + lo, seg),
                            )
                    else:
                        eng().dma_start(
                            AP(t[:].tensor, 0, lblk),
                            AP(srcd, g * 128 * IT, lblk),
                        )

                o1t = tout.tile([128, OF3], BF16, tag="o1t")
                o2t = tout.tile([128, OF3], BF16, tag="o2t")
                tkh, tqh = tk[:].tensor, tq[:].tensor

                def mul(meng, ot, a, b, j, t0=0, fd=T3):
                    sub = [[OF3, 128], [9, fd // 9], [1, 9]]
                    isub = [[IT, 128], [9, fd // 9], [1, 9]]
                    bsub = [[IT, 128], [9, fd // 9], [0, 9]]
                    meng.tensor_mul(
                        AP(ot[:].tensor, j * T3 + t0, sub),
                        AP(a, 2 - j + t0, isub),
                        AP(b, 6 - j + t0, bsub),
                    )

                # all muls on DVE: gpsimd tensor ops contend on the
                # shared SBUF port and dilate concurrent DVE muls ~2.6x
                # (measured), so a second compute engine is a net loss.
                def store(od, ot, lo, n):
                    seg = [[OF3, 128], [1, n]]
                    eng().dma_start(
                        AP(od, g * 128 * OF3 + lo, seg),
                        AP(ot[:].tensor, lo, seg),
                    )

                if g == NG - 1:
                    # last group: interleave outputs and store each span
                    # as soon as computed so only one 0.79 MB span store
                    # trails the final mul
                    mul(nc.vector, o1t, tkh, tqh, 0)
                    mul(nc.vector, o2t, tqh, tkh, 0)
                    store(o1, o1t, 0, T3)
                    store(o2, o2t, 0, T3)
                    mul(nc.vector, o1t, tkh, tqh, 1)
                    mul(nc.vector, o2t, tqh, tkh, 1)
                    store(o1, o1t, T3, T3)
                    store(o2, o2t, T3, T3)
                    mul(nc.vector, o1t, tkh, tqh, 2)
                    store(o1, o1t, 2 * T3, T3)
                    mul(nc.vector, o2t, tqh, tkh, 2)
                    store(o2, o2t, 2 * T3, T3)
                else:
                    if g == 0:
                        # both span-0 left-pieces need only the left
                        # half-loads; the right halves land meanwhile
                        mul(nc.vector, o1t, tkh, tqh, 0, 0, FDL)
                        mul(nc.vector, o2t, tqh, tkh, 0, 0, FDL)
                        mul(nc.vector, o1t, tkh, tqh, 0, FDL, T3 - FDL)
                        mul(nc.vector, o1t, tkh, tqh, 1)
                        mul(nc.vector, o1t, tkh, tqh, 2)
                        store(o1, o1t, 0, OF3)
                        mul(nc.vector, o2t, tqh, tkh, 0, FDL, T3 - FDL)
                        for j in (1, 2):
                            mul(nc.vector, o2t, tqh, tkh, j)
                        store(o2, o2t, 0, OF3)
                    else:
                        for j in range(3):
                            mul(nc.vector, o1t, tkh, tqh, j)
                        store(o1, o1t, 0, OF3)
                        for j in range(3):
                            mul(nc.vector, o2t, tqh, tkh, j)
                        store(o2, o2t, 0, OF3)
    _split_waits(nc)
    return nc


_NC_CACHE = []


def _get_nc():
    if not _NC_CACHE:
        _NC_CACHE.append(_build_program())
    return _NC_CACHE[0]


# ---- host-side index maps (channel-independent, computed once) ------------

def _index_maps():
    starts = np.asarray(STARTS)
    offs = np.asarray(OFFJ)
    t = np.arange(T3)
    n_map = starts[:, None, None] + offs[None, :, None] + t[None, None, :]
    c4 = (n_map // 9) * 9 + 4
    q = n_map // 128
    i = q % 128
    qc = c4 // 128
    ic = qc % 128
    third = n_map // 49152
    valid = np.zeros(n_map.shape, bool)
    valid[:, 0, :] = third[:, 0, :] == 0
    valid[:, 1, :] = (third[:, 1, :] == 1) & (i[:, 1, :] <= 126) & (ic[:, 1, :] <= 126)
    valid[:, 2, :] = (third[:, 2, :] == 2) & (i[:, 2, :] <= 125) & (ic[:, 2, :] <= 125)
    covered = np.zeros(L9, bool)
    covered[n_map[valid]] = True
    patch = np.nonzero(~covered)[0]
    return n_map.reshape(-1), valid.reshape(-1), patch


N_MAP, VALID, PATCH = _index_maps()
N_VALID = N_MAP[VALID]
C4_FULL = (np.arange(L9) // 9) * 9 + 4
PATCH_C4 = C4_FULL[PATCH]


def _raws(x):
    """[B,C,H,W] -> [BC, 9L] f32 patch-major unfold (RAW)."""
    x = np.ascontiguousarray(np.asarray(x, dtype=np.float32).reshape(BC, H, W))
    pad = np.pad(x, ((0, 0), (1, 1), (1, 1)))
    return np.stack(
        [pad[:, di : di + H, dj : dj + W] for di in range(3) for dj in range(3)],
        axis=1,
    ).reshape(BC, L9)


def _pretile(raw16):
    """[BC, 9L] bf16 -> [N_CORES, NG, 128, IT] tiles (first third + halo)."""
    rpad = np.concatenate(
        [np.zeros((BC, 2), raw16.dtype), raw16[:, : STARTS[-1] + IT]], axis=1
    )
    idx = np.asarray(STARTS)[:, None] + np.arange(IT)[None, :]   # [16, IT]
    tiles = rpad[:, idx]                                         # [BC, 16, IT]
    return np.ascontiguousarray(tiles.reshape(N_CORES, NG, 128, IT))


_HOST_STATE = {}


def make_in_maps(key_map, query_map):
    rk = _raws(key_map)
    rq = _raws(query_map)
    rk16 = rk.astype(NP_BF16)
    rq16 = rq.astype(NP_BF16)
    _HOST_STATE["rk"] = rk
    _HOST_STATE["rq"] = rq
    kb = _pretile(rk16)
    qb = _pretile(rq16)
    return [{"kp": kb[m], "qp": qb[m]} for m in range(N_CORES)]


def assemble(results):
    rk, rq = _HOST_STATE["rk"], _HOST_STATE["rq"]
    outs = []
    for name, ra, rb in (("o1", rk, rq), ("o2", rq, rk)):
        arr = np.stack([np.asarray(results[m][name]) for m in range(N_CORES)])
        # [core, g, p=(chl,s), 3*T3] -> [BC, 16*3*T3]
        arr = arr.reshape(N_CORES, NG, NCH, SPC * OF3).reshape(BC, SPC * OF3)
        out = np.empty((BC, L9), np.float32)
        out[:, N_VALID] = arr[:, VALID].astype(np.float32)
        # exact f32 host patch for identity-invalid / uncovered positions
        out[:, PATCH] = ra[:, PATCH] * rb[:, PATCH_C4]
        outs.append(out.reshape(B, C, L, 9))
    return tuple(outs)


def kernel(key_map, query_map):
    nc = _get_nc()
    in_maps = make_in_maps(key_map, query_map)
    res = run_bass_kernel_spmd(nc, in_maps, core_ids=list(range(N_CORES)))
    return assemble(res.results)


# revision 25
# speedup vs baseline: 1.1828x; 1.1828x over previous
"""Trainium2 Bass kernel for the sparse-attention (local 3x3 unfold) problem.

Semantics (per channel; the reference's .reshape is a RAW reinterpretation
of [9, L] -> [L, 9], so with RAW = patch-major unfold flattened [9*L]):
  out1_flat[n] = RAW_k[n] * RAW_q[9*(n//9) + 4]     (n in [0, 9L))
  out2_flat[n] = RAW_q[n] * RAW_k[9*(n//9) + 4]

Key identity (v10): RAW[n] == RAW[n + 49024] == RAW[n + 98048] wherever
the patch-row shift stays in range (shifting patch p by +3 keeps the same
column-variant dj while di absorbs the row shift).  So the device loads
only the FIRST THIRD of RAW (~6.3 MB/core instead of 18.9 MB) and each
partition computes all three thirds from one tile via three shifted
access patterns:

  partition s owns spans  n in [start_s + j*49023, + 3078), j = 0,1,2
  with start_s = 3072*s - (3*s mod 9)  (multiple of 9, uniform T=3078,
  tiny overlaps), tile[x] = RAW[start_s - 2 + x] (x in [0,3080)):
    in0 tile-offset = (2-j) + t ;  center tile-offset = (6-j) + 9*(t//9)

The ~0.8% of outputs where the identity breaks (patch rows i=127/126,
center spill, coverage gaps) are computed exactly in f32 on the host
during unscramble (precomputed index masks, channel-independent).

All device traffic is bf16 (gate is 2e-2; bf16 in+product is ~0.6%).
DMA: ~6.3 MB loads + ~19 MB stores per core, all [[W,128],[1,W]] blocks
with 6.2 KB descriptors; vector engine (~81 us, 1x mode) is the wall.

Sharding: pure data-parallel over the 256 (b,c) channels; 32 per core.
"""

import sys

for _p in ("/opt/trn_rl_repo", "/opt/pypackages"):
    if _p not in sys.path:
        sys.path.insert(0, _p)

import numpy as np

import concourse.bass as bass
import concourse.mybir as mybir
import concourse.tile as tile
from concourse.bass import AP
from concourse.bass_utils import run_bass_kernel_spmd
from concourse.vector_clock import ScopedClock

# ---------------------------------------------------------------------------
# Patch: this container's walrus rejects >1 sync-wait on the Tile tail Drain
# ("Too many sync wait commands").  Spill extra waits onto SP NOPs, which
# execute in program order before the all-engine barrier, preserving the
# "all work done before sem clear" semantics.
# ---------------------------------------------------------------------------


def _drain_and_barrier(self, tick_clock, wait_clock):
    nc = self.nc
    drain_inst = nc.sync.drain()
    wait_clock.add_sem_waits(
        drain_inst.ins, ScopedClock({None: tick_clock.global_clock})
    )
    si = drain_inst.ins.sync_info
    if si is not None and len(si.on_wait) > 1:
        waits = list(si.on_wait)
        drain_inst.ins.sync_info = mybir.SyncInfo(
            on_wait=waits[:1], on_update=list(si.on_update)
        )
        for w in waits[1:]:
            nop = nc.sync.nop(nofuse=True)
            nop.ins.sync_info = mybir.SyncInfo(on_wait=[w], on_update=[])

    nc.all_engine_barrier()
    assert self.sems is not None
    popped = nc._tile_sem_poison_stack.pop()
    assert popped is self._sem_poison
    nc.clear_and_free_semaphores(list(self.sems.allocated().values()))
    nc.all_engine_barrier()


tile.TileContext._drain_and_barrier = _drain_and_barrier


def _split_waits(nc, maxw=1):
    """Walrus here allows only `maxw` sync-waits per instruction: move extra
    waits onto same-engine NOPs inserted immediately before the instruction
    (same engine stream => executes before it)."""
    for fn in nc.m.functions:
        for bb in fn.blocks:
            out = []
            for inst in bb.instructions:
                si = getattr(inst, "sync_info", None)
                if si is not None and len(si.on_wait) > maxw:
                    waits = list(si.on_wait)
                    for w in waits[:-maxw]:
                        nop = mybir.InstNoOp(
                            name=nc.get_next_instruction_name(),
                            bass_nofuse=True,
                        )
                        nop.engine = inst.engine
                        nop.sync_info = mybir.SyncInfo(on_wait=[w], on_update=[])
                        nc.register_instruction(nop)
                        out.append(nop)
                    inst.sync_info = mybir.SyncInfo(
                        on_wait=waits[-maxw:], on_update=list(si.on_update)
                    )
                out.append(inst)
            bb.instructions[:] = out

# ---------------------------------------------------------------------------

F32 = mybir.dt.float32
BF16 = mybir.dt.bfloat16
NP_BF16 = mybir.dt.np(mybir.dt.bfloat16)

N_CORES = 8
B, C, H, W = 4, 64, 128, 128
BC = B * C                # 256 channels
CPC = BC // N_CORES       # 32 channels per core
NG = 4                    # channel groups per core
NCH = CPC // NG           # 8 channels per group
SPC = 16                  # partitions per channel
L = H * W
L9 = 9 * L                # 147456 flats per channel
DD = 49024                # RAW periodicity: RAW[n] == RAW[n+DD] (in range)
T3 = 3078                 # span length (multiple of 9)
IT = 3080                 # input tile width (span + 2-elem halo)
OF3 = 3 * T3              # 9234: output tile width (3 spans)
STARTS = [3072 * s - ((3 * s) % 9) for s in range(SPC)]
OFFJ = [0, DD - 1, 2 * DD - 2]   # span offsets (multiples of 9)


def _build_program():
    nc = bass.Bass(trn_type="TRN2")
    kp = nc.dram_tensor("kp", [NG, 128, IT], BF16, kind="ExternalInput")
    qp = nc.dram_tensor("qp", [NG, 128, IT], BF16, kind="ExternalInput")
    o1 = nc.dram_tensor("o1", [NG, 128, OF3], BF16, kind="ExternalOutput")
    o2 = nc.dram_tensor("o2", [NG, 128, OF3], BF16, kind="ExternalOutput")

    # DMA on the two HWDGE queues (16 fat instructions, ~31us DGE each);
    # gpsimd computes one multiply per group (issued first, hidden under
    # the DVE chain), cutting the DVE wall from ~81us to ~67us.
    engines = [nc.sync, nc.scalar]
    eng_i = [0]

    def eng():
        e = engines[eng_i[0] % len(engines)]
        eng_i[0] += 1
        return e

    lblk = [[IT, 128], [1, IT]]
    sblk = [[OF3, 128], [1, OF3]]
    full = [[OF3, 128], [9, T3 // 9], [1, 9]]
    infull = [[IT, 128], [9, T3 // 9], [1, 9]]
    bcast = [[IT, 128], [9, T3 // 9], [0, 9]]
    with tile.TileContext(nc) as tc:
        with (
            tc.tile_pool(name="tin", bufs=3) as tin,
            tc.tile_pool(name="tout", bufs=2) as tout,
        ):
            HSPLIT = 1552             # g0 load split point (free dim)
            FDL = 1539                # left-piece mul FD (= 171*9)
            h2 = [[OF3, 128], [1, 2 * T3]]
            t1 = [[OF3, 128], [1, T3]]
            for g in range(NG):
                tk = tin.tile([128, IT], BF16, tag="tk")
                tq = tin.tile([128, IT], BF16, tag="tq")
                for srcd, t in ((kp, tk), (qp, tq)):
                    if g == 0:
                        # split the very first loads so the first mul can
                        # start on the left half-tile ~3us earlier
                        for lo, hi in ((0, HSPLIT), (HSPLIT, IT)):
                            seg = [[IT, 128], [1, hi - lo]]
                            eng().dma_start(
                                AP(t[:].tensor, lo, seg),
                                AP(srcd, g * 128 * IT + lo, seg),
                            )
                    else:
                        eng().dma_start(
                            AP(t[:].tensor, 0, lblk),
                            AP(srcd, g * 128 * IT, lblk),
                        )

                o1t = tout.tile([128, OF3], BF16, tag="o1t")
                o2t = tout.tile([128, OF3], BF16, tag="o2t")
                tkh, tqh = tk[:].tensor, tq[:].tensor

                def mul(meng, ot, a, b, j, t0=0, fd=T3):
                    sub = [[OF3, 128], [9, fd // 9], [1, 9]]
                    isub = [[IT, 128], [9, fd // 9], [1, 9]]
                    bsub = [[IT, 128], [9, fd // 9], [0, 9]]
                    meng.tensor_mul(
                        AP(ot[:].tensor, j * T3 + t0, sub),
                        AP(a, 2 - j + t0, isub),
                        AP(b, 6 - j + t0, bsub),
                    )

                # all muls on DVE: gpsimd tensor ops contend on the
                # shared SBUF port and dilate concurrent DVE muls ~2.6x
                # (measured), so a second compute engine is a net loss.
                # all muls on DVE: gpsimd tensor ops contend on the
                # shared SBUF port and dilate concurrent DVE muls ~2.6x
                # (measured), so a second compute engine is a net loss.
                if g == NG - 1:
                    # last group: interleave outputs and split stores so
                    # only one 0.79 MB span store trails the final mul
                    mul(nc.vector, o1t, tkh, tqh, 0)
                    mul(nc.vector, o2t, tqh, tkh, 0)
                    mul(nc.vector, o1t, tkh, tqh, 1)
                    eng().dma_start(
                        AP(o1, g * 128 * OF3, h2),
                        AP(o1t[:].tensor, 0, h2),
                    )
                    mul(nc.vector, o1t, tkh, tqh, 2)
                    eng().dma_start(
                        AP(o1, g * 128 * OF3 + 2 * T3, t1),
                        AP(o1t[:].tensor, 2 * T3, t1),
                    )
                    mul(nc.vector, o2t, tqh, tkh, 1)
                    eng().dma_start(
                        AP(o2, g * 128 * OF3, h2),
                        AP(o2t[:].tensor, 0, h2),
                    )
                    mul(nc.vector, o2t, tqh, tkh, 2)
                    eng().dma_start(
                        AP(o2, g * 128 * OF3 + 2 * T3, t1),
                        AP(o2t[:].tensor, 2 * T3, t1),
                    )
                else:
                    if g == 0:
                        mul(nc.vector, o1t, tkh, tqh, 0, 0, FDL)
                        mul(nc.vector, o1t, tkh, tqh, 0, FDL, T3 - FDL)
                    else:
                        mul(nc.vector, o1t, tkh, tqh, 0)
                    for j in (1, 2):
                        mul(nc.vector, o1t, tkh, tqh, j)
                    eng().dma_start(
                        AP(o1, g * 128 * OF3, sblk),
                        AP(o1t[:].tensor, 0, sblk),
                    )
                    for j in range(3):
                        mul(nc.vector, o2t, tqh, tkh, j)
                    eng().dma_start(
                        AP(o2, g * 128 * OF3, sblk),
                        AP(o2t[:].tensor, 0, sblk),
                    )
    _split_waits(nc)
    return nc


_NC_CACHE = []


def _get_nc():
    if not _NC_CACHE:
        _NC_CACHE.append(_build_program())
    return _NC_CACHE[0]


# ---- host-side index maps (channel-independent, computed once) ------------

def _index_maps():
    starts = np.asarray(STARTS)
    offs = np.asarray(OFFJ)
    t = np.arange(T3)
    n_map = starts[:, None, None] + offs[None, :, None] + t[None, None, :]
    c4 = (n_map // 9) * 9 + 4
    q = n_map // 128
    i = q % 128
    qc = c4 // 128
    ic = qc % 128
    third = n_map // 49152
    valid = np.zeros(n_map.shape, bool)
    valid[:, 0, :] = third[:, 0, :] == 0
    valid[:, 1, :] = (third[:, 1, :] == 1) & (i[:, 1, :] <= 126) & (ic[:, 1, :] <= 126)
    valid[:, 2, :] = (third[:, 2, :] == 2) & (i[:, 2, :] <= 125) & (ic[:, 2, :] <= 125)
    covered = np.zeros(L9, bool)
    covered[n_map[valid]] = True
    patch = np.nonzero(~covered)[0]
    return n_map.reshape(-1), valid.reshape(-1), patch


N_MAP, VALID, PATCH = _index_maps()
N_VALID = N_MAP[VALID]
C4_FULL = (np.arange(L9) // 9) * 9 + 4
PATCH_C4 = C4_FULL[PATCH]


def _raws(x):
    """[B,C,H,W] -> [BC, 9L] f32 patch-major unfold (RAW)."""
    x = np.ascontiguousarray(np.asarray(x, dtype=np.float32).reshape(BC, H, W))
    pad = np.pad(x, ((0, 0), (1, 1), (1, 1)))
    return np.stack(
        [pad[:, di : di + H, dj : dj + W] for di in range(3) for dj in range(3)],
        axis=1,
    ).reshape(BC, L9)


def _pretile(raw16):
    """[BC, 9L] bf16 -> [N_CORES, NG, 128, IT] tiles (first third + halo)."""
    rpad = np.concatenate(
        [np.zeros((BC, 2), raw16.dtype), raw16[:, : STARTS[-1] + IT]], axis=1
    )
    idx = np.asarray(STARTS)[:, None] + np.arange(IT)[None, :]   # [16, IT]
    tiles = rpad[:, idx]                                         # [BC, 16, IT]
    return np.ascontiguousarray(tiles.reshape(N_CORES, NG, 128, IT))


_HOST_STATE = {}


def make_in_maps(key_map, query_map):
    rk = _raws(key_map)
    rq = _raws(query_map)
    rk16 = rk.astype(NP_BF16)
    rq16 = rq.astype(NP_BF16)
    _HOST_STATE["rk"] = rk
    _HOST_STATE["rq"] = rq
    kb = _pretile(rk16)
    qb = _pretile(rq16)
    return [{"kp": kb[m], "qp": qb[m]} for m in range(N_CORES)]


def assemble(results):
    rk, rq = _HOST_STATE["rk"], _HOST_STATE["rq"]
    outs = []
    for name, ra, rb in (("o1", rk, rq), ("o2", rq, rk)):
        arr = np.stack([np.asarray(results[m][name]) for m in range(N_CORES)])
        # [core, g, p=(chl,s), 3*T3] -> [BC, 16*3*T3]
        arr = arr.reshape(N_CORES, NG, NCH, SPC * OF3).reshape(BC, SPC * OF3)
        out = np.empty((BC, L9), np.float32)
        out[:, N_VALID] = arr[:, VALID].astype(np.float32)
        # exact f32 host patch for identity-invalid / uncovered positions
        out[:, PATCH] = ra[:, PATCH] * rb[:, PATCH_C4]
        outs.append(out.reshape(B, C, L, 9))
    return tuple(outs)


def kernel(key_map, query_map):
    nc = _get_nc()
    in_maps = make_in_maps(key_map, query_map)
    res = run_bass_kernel_spmd(nc, in_maps, core_ids=list(range(N_CORES)))
    return assemble(res.results)
